# revision 54
# baseline (speedup 1.0000x reference)
"""Trainium2 Bass kernel for nn_BasisAffinityGAT (B=8, N=512, D=R=128, K=8).

Math (matches reference.py):
    fused = concat(desc, nve) @ W_fuse + b_fuse                 [B,N,D]
    q = fused @ W_q[k];  kk = fused @ W_k[k]                    per basis
    e_q[b,k,n] = lrelu(q).a_q[k];  e_k likewise
    logits = e_q[:,:,:,None] + e_k[:,:,None,:], symmetrized
    alpha  = softmax(logits, -1); ema update; bias_log = log(clip(ema'))

Exact algebra used:
  * sym-logits[i,j] = 0.5*(s_i + s_j) with s = e_q + e_k, so the row
    softmax collapses: alpha[b,k,i,j] = softmax_j(0.5*s[b,k,:])[j],
    independent of i.
  * lrelu(x) = 0.6*x + 0.4*|x| (slope 0.2), so
    0.5*s[b,k,n] = fused[b,n,:] @ wlin[:,k]
                   + 0.2*(a_q[k] . |q_T|) + 0.2*(a_k[k] . |k_T|)
    with wlin[:,k] = 0.3*(W_q[k] @ a_q[k] + W_k[k] @ a_k[k]) host-folded.
  * b_fuse enters only through |q+bq|, |k+bk| (bq = b_fuse@W_q etc, applied
    as ACT per-partition bias) and an additive constant in s that the
    softmax cancels, so `fused` is computed WITHOUT the bias on device.
  * bias_log content is batch-independent ([K,N,N] broadcast over B).

Sharding: core m owns basis k=m for ALL batches (K-sharded, SPMD, zero
cross-core communication; collectives have ~78us launch latency here).

Performance structure (memory-regime problem: 16.8 MB of output writes
per core dominates; per-core DMA ~360 GB/s shared by reads+writes, so
T ~ first_write_time + write_bytes/rate once the queues stay packed):
  * x and ema are loaded as bf16 (host-cast), halving input read
    bytes; all weight matmuls run in bf16 (rel err ~4e-3, gate 2e-2).
  * Queue plan: qSync HWDGE carries all input reads issued upfront
    (xb bufs=8, ema serialized last on the same queue so its packets
    never compete with x) plus the odd-batch alpha writes and 3 bias
    chunks; qScalar HWDGE carries the weight pack, even-batch alpha
    writes and 2 bias chunks; gpsimd carries 3 bias chunks. Alpha
    issues are deferred by one batch because a dma_start stalls its
    issuing engine until the source tile is ready — an inline issue
    would block the ACT abs/exp cadence for the whole softmax chain.
  * SWDGE + partition-broadcast ucode have ~10us first-use latency;
    tiny warm-up ops at t=0 hide it.
  * p_b is normalized on one partition (exp/sum, DVE) and replicated
    to 128 partitions with gpsimd partition_broadcast — taking the
    replication matmul off the PE (the per-batch cadence limiter) and
    the PSUM->SBUF copy off the DVE. pbar accumulates on a [1,N]
    vector and is broadcast once at the end the same way; the bias
    tail is 4 independently-buffered full-width chunks (2KB
    descriptors — measurably faster than 1KB half-width ones) spread
    over all three queues so it drains in parallel.
"""

import sys

import numpy as np

if "/opt/trn_rl_repo" not in sys.path:
    sys.path.insert(0, "/opt/trn_rl_repo")

from contextlib import ExitStack

import ml_dtypes

import concourse.bass as bass
import concourse.tile as tile
from concourse import bacc, mybir
from concourse.bass_utils import run_bass_kernel_spmd

B, N, D, K = 8, 512, 128, 8
R = D
MOM = 0.99
EPS = 1e-6
N_CORES = 8
F32 = mybir.dt.float32
F32R = mybir.dt.float32r
BF16 = mybir.dt.bfloat16
AF = mybir.ActivationFunctionType
ALU = mybir.AluOpType
NPBF = ml_dtypes.bfloat16

WCOLS = 2 * R + 3  # wq | wk | 0.2*aq | 0.2*ak | wlin


def build():
    """Build the SPMD per-core Bass program (identical on all 8 cores)."""
    nc = bacc.Bacc("TRN2", target_bir_lowering=False, debug=False,
                   num_devices=N_CORES)

    # ---- per-core external tensors -------------------------------------
    # xT[b,h,d,n]: h=0 desc[b].T, h=1 nve[b].T  (bf16, same on all cores)
    xT = nc.dram_tensor("xT", [B, 2, D, N], BF16, kind="ExternalInput")
    wfuse = nc.dram_tensor("wfuse", [D, 2 * D], BF16, kind="ExternalInput")
    wpack = nc.dram_tensor("wpack", [D, WCOLS], BF16, kind="ExternalInput")
    bcols = nc.dram_tensor("bcols", [D, 2], F32, kind="ExternalInput")
    ema = nc.dram_tensor("ema", [N, N], BF16, kind="ExternalInput")  # [m]
    alpha = nc.dram_tensor("alpha", [B, N, N], F32, kind="ExternalOutput")
    biaso = nc.dram_tensor("bias", [B, N, N], F32, kind="ExternalOutput")

    with ExitStack() as ctx:
        tc = ctx.enter_context(tile.TileContext(nc))
        const = ctx.enter_context(tc.tile_pool(name="const", bufs=1))
        work = ctx.enter_context(tc.tile_pool(name="work", bufs=2))
        absp = ctx.enter_context(tc.tile_pool(name="absp", bufs=4))
        psum = ctx.enter_context(tc.tile_pool(name="psum", bufs=1, space="PSUM"))

        wfuse_sb = const.tile([D, 2 * D], BF16)
        wpack_sb = const.tile([D, WCOLS], BF16)
        bcols_sb = const.tile([D, 2], F32)
        ema_sb = const.tile([128, 4 * N], BF16)
        ones1_sb = const.tile([1, D], F32)
        pbs1 = const.tile([1, N], F32)
        warm_sb = const.tile([1, 16], BF16)
        warm2_sb = const.tile([1, 16], F32)
        warm3_sb = const.tile([8, 16], F32)

        # ---- input DMAs, issued back-to-back upfront -------------------
        # SWDGE (gpsimd DMA) has ~10us first-use latency (Q7 ucode lib
        # load); fire a tiny dummy DMA immediately so the queue is warm
        # when the bias writes need it near the end. Same for the
        # partition-broadcast ucode used on the alpha path.
        nc.gpsimd.dma_start(warm_sb[:], wfuse[0:1, 0:16])
        nc.vector.memset(warm2_sb[:], 0.0)
        nc.gpsimd.partition_broadcast(warm3_sb[:], warm2_sb[:], 8)
        # qSync: wfuse gates the first matmul; x loaded as 4 pair-DMAs
        # (each ~0.65us of SP issue time paces the queue, so fewer and
        # bigger reads win); ema last — serialized behind x on the same
        # queue so its packets never compete with the x reads.
        nc.sync.dma_start(wfuse_sb[:], wfuse[:])
        xbs = []
        for b in range(B):
            xb = work.tile([D, 2 * N], BF16, tag="xb", bufs=8)
            nc.sync.dma_start(
                xb[:].rearrange("d (h n) -> d h n", h=2),
                xT[b].rearrange("h d n -> d h n"))
            xbs.append(xb)
        nc.sync.dma_start(
            ema_sb[:].rearrange("p (i n) -> p i n", i=4),
            ema.ap().rearrange("(p i) n -> p i n", p=128))
        # qScalar: small weights (needed from the first q matmul on)
        nc.scalar.dma_start(wpack_sb[:], wpack[:])
        nc.scalar.dma_start(bcols_sb[:], bcols[:])
        nc.vector.memset(ones1_sb[:], 1.0)

        wq_ap = wpack_sb[:, 0:R]
        wk_ap = wpack_sb[:, R:2 * R]
        aq_ap = wpack_sb[:, 2 * R:2 * R + 1]
        ak_ap = wpack_sb[:, 2 * R + 1:2 * R + 2]
        wlin_ap = wpack_sb[:, 2 * R + 2:2 * R + 3]
        bq_ap = bcols_sb[:, 0:1]
        bk_ap = bcols_sb[:, 1:2]

        pending_alpha = []
        for b in range(B):
            xb = xbs[b]
            psum_f = psum.tile([D, N], F32, tag="mm", bufs=3)
            nc.tensor.matmul(psum_f[:], wfuse_sb[:, 0:D], xb[:, 0:N],
                             start=True, stop=False)
            nc.tensor.matmul(psum_f[:], wfuse_sb[:, D:2 * D],
                             xb[:, N:2 * N], start=False, stop=True)
            fused_sb = absp.tile([D, N], BF16, tag="fused", bufs=3)
            nc.vector.tensor_copy(fused_sb[:], psum_f[:])
            psum_s = psum.tile([1, N], F32, tag="ps", bufs=2)
            nc.tensor.matmul(psum_s[:], wlin_ap, fused_sb[:],
                             start=True, stop=False)
            psum_q = psum.tile([D, N], F32, tag="mm", bufs=3)
            nc.tensor.matmul(psum_q[:], wq_ap, fused_sb[:],
                             start=True, stop=True)
            absq = absp.tile([D, N], BF16, tag="abs", bufs=4)
            nc.scalar.activation(absq[:], psum_q[:], AF.Abs, bias=bq_ap)
            nc.tensor.matmul(psum_s[:], aq_ap, absq[:],
                             start=False, stop=False)
            psum_k = psum.tile([D, N], F32, tag="mm", bufs=3)
            nc.tensor.matmul(psum_k[:], wk_ap, fused_sb[:],
                             start=True, stop=True)
            absk = absp.tile([D, N], BF16, tag="abs", bufs=4)
            nc.scalar.activation(absk[:], psum_k[:], AF.Abs, bias=bk_ap)
            nc.tensor.matmul(psum_s[:], ak_ap, absk[:],
                             start=False, stop=True)

            # ---- softmax over free dim (no max-shift: |s| is O(3)) -----
            expv = work.tile([1, N], F32, tag="ex", bufs=4)
            sume = work.tile([1, 1], F32, tag="se", bufs=4)
            nc.scalar.activation(expv[:], psum_s[:], AF.Exp,
                                 scale=1.0, accum_out=sume[:])
            rsum = work.tile([1, 1], F32, tag="rs", bufs=4)
            nc.vector.reciprocal(rsum[:], sume[:])

            # ---- alpha[b, i, :] = p_b for all i ------------------------
            # p = exp/sum on one partition, then gpsimd partition-
            # broadcast to 128 (takes the whole replication off the PE
            # and the psum->sbuf copy off the DVE).
            pnorm = work.tile([1, N], F32, tag="pn", bufs=4)
            nc.vector.tensor_scalar_mul(pnorm[:], expv[:], rsum[:])
            rep_t = absp.tile([128, N], F32, tag="repsb", bufs=4)
            nc.gpsimd.partition_broadcast(rep_t[:], pnorm[:], 128)
            # pbar partial sum on [1,N] (broadcast once at the end)
            if b == 0:
                nc.vector.tensor_copy(pbs1[:], pnorm[:])
            else:
                nc.vector.tensor_add(pbs1[:], pnorm[:], pbs1[:])
            src = rep_t[:].rearrange(
                "p (o n) -> p o n", o=1).broadcast_to([128, 4, N])
            dst = alpha[b].rearrange("(p i) j -> p i j", p=128)
            # HWDGE queues only (SWDGE-carried alpha writes pace the whole
            # pipeline). The issue is deferred by one batch: a dma_start
            # stalls its issuing engine until the source tile is ready, so
            # issuing rep_t[b] inline would block the ACT stream for the
            # whole softmax chain. Batch 0 issues inline on the sync
            # engine instead — it is idle after the read issues, so the
            # stall is free and the first write starts earlier.
            if b == 0:
                nc.sync.dma_start(dst, src)
            else:
                pending_alpha.append((b, dst, src))
                if len(pending_alpha) > 1:
                    pb_, pdst, psrc = pending_alpha.pop(0)
                    (nc.scalar if pb_ % 2 == 0 else nc.sync).dma_start(
                        pdst, psrc)
        for pb_, pdst, psrc in pending_alpha:
            (nc.scalar if pb_ % 2 == 0 else nc.sync).dma_start(pdst, psrc)

        # ---- bias_log tail ---------------------------------------------
        # pb_sb[p,n] = sum_b p_b[n] on every partition (gpsimd bcast);
        # C = (1-MOM)/(B*MOM); bias = ln(MOM * max(ema + C*pb, EPS/MOM))
        pb_sb = absp.tile([128, N], F32, tag="pbb", bufs=1)
        nc.gpsimd.partition_broadcast(pb_sb[:], pbs1[:], 128)
        # 4 full-width chunks, independently buffered so the three
        # queues drain the tail in parallel instead of serializing on
        # tile reuse.
        # Full-width chunks -> 2KB descriptors (vs 1KB at half-width).
        bias_q = [nc.scalar, nc.sync, nc.gpsimd, nc.sync]
        dst_all = biaso.ap().rearrange("b (p i) j -> i p b j", i=4)
        for i in range(4):
            sl = slice(i * N, (i + 1) * N)
            u = work.tile([128, N], F32, tag="u", bufs=4)
            nc.vector.scalar_tensor_tensor(
                u[:], pb_sb[:], 0.01 / B / MOM,
                ema_sb[:, sl], op0=ALU.mult, op1=ALU.add)
            v = work.tile([128, N], F32, tag="v", bufs=4)
            nc.vector.tensor_scalar_max(v[:], u[:], EPS / MOM)
            bias_t = work.tile([128, N], F32, tag="biassb", bufs=4)
            nc.scalar.activation(bias_t[:], v[:], AF.Ln, scale=MOM)
            src = bias_t[:].rearrange(
                "p (o j) -> p o j", o=1).broadcast_to([128, B, N])
            bias_q[i].dma_start(dst_all[i], src)

    nc.compile()
    return nc


_NC_CACHE = None


def _get_nc():
    global _NC_CACHE
    if _NC_CACHE is None:
        _NC_CACHE = build()
    return _NC_CACHE


def make_in_maps(desc_embeddings, name_value_embeddings, W_fuse, b_fuse,
                 W_q, W_k, a, alpha_ema):
    """Host-side sharding / weight prep -> per-core input dicts."""
    desc = np.asarray(desc_embeddings, np.float32)
    nve = np.asarray(name_value_embeddings, np.float32)
    W_fuse = np.asarray(W_fuse, np.float32)
    b_fuse = np.asarray(b_fuse, np.float32)
    W_q = np.asarray(W_q, np.float32)
    W_k = np.asarray(W_k, np.float32)
    a = np.asarray(a, np.float32)
    alpha_ema = np.asarray(alpha_ema, np.float32)

    a_q = a[:, :R, 0]                      # [K,R]
    a_k = a[:, R:, 0]                      # [K,R]
    wlin = 0.3 * (np.einsum("kdr,kr->kd", W_q, a_q)
                  + np.einsum("kdr,kr->kd", W_k, a_k))  # [K,D]
    bq = np.einsum("d,kdr->kr", b_fuse, W_q)            # [K,R]
    bk = np.einsum("d,kdr->kr", b_fuse, W_k)            # [K,R]

    # xT[b] = [desc[b].T, nve[b].T] in bf16 — shared across cores
    xT = np.ascontiguousarray(
        np.stack([np.stack([desc[b].T, nve[b].T], axis=0)
                  for b in range(B)], axis=0)).astype(NPBF)
    # wfuse_sb[c, h*D+d] = W_fuse[h*D+c, d]
    wfuse_t = np.ascontiguousarray(
        W_fuse.reshape(2, D, D).transpose(1, 0, 2).reshape(D, 2 * D)
    ).astype(NPBF)

    shared = dict(xT=xT, wfuse=wfuse_t)
    in_maps = []
    for m in range(N_CORES):
        wp = np.concatenate(
            [W_q[m], W_k[m], 0.2 * a_q[m][:, None], 0.2 * a_k[m][:, None],
             wlin[m][:, None]], axis=1)
        in_maps.append(dict(
            shared,
            wpack=np.ascontiguousarray(wp).astype(NPBF),
            bcols=np.ascontiguousarray(
                np.stack([bq[m], bk[m]], axis=1).astype(np.float32)),
            ema=np.ascontiguousarray(alpha_ema[m]).astype(NPBF)))
    return in_maps


def gather(results):
    alpha_full = np.stack([r["alpha"] for r in results], axis=1)
    bias_full = np.stack([r["bias"] for r in results], axis=1)
    return bias_full, alpha_full


def kernel(**inputs):
    nc = _get_nc()
    in_maps = make_in_maps(**inputs)
    res = run_bass_kernel_spmd(nc, in_maps, list(range(N_CORES)))
    return gather(res.results)
